# revision 5
# baseline (speedup 1.0000x reference)
"""Trainium2 Bass kernel: BiasFreeLayerNorm + MQA attention + out-proj.

Problem (nn_Attention_90812788506696):
  x[B=4, N=2048, C=1024]; std over C (ddof=1, no mean subtraction of x);
  xn = x/(std+eps)*gamma; q = xn@Wq.T (16 heads x 64); k,v = x@Wkv.T (1 shared
  kv head, MQA); softmax(q k^T / sqrt(64)) @ v; concat; @Wo.T; * ls_scale.

Sharding (8 cores): core = (batch b = core//2, head-group g = core%2 of 8
query heads). K/V replicated per batch. Each core produces a PARTIAL
y_part[b] = attn_out(8 heads) @ Wo[:, g-slice].T (ls folded); host sums the
two partials per batch. No device collectives.

Device dataflow per core (feature-major layout; "T" = [features, tokens]):
  - stream xT slices; LN stats via ones-matmul (sum_c x, sum_c x^2) -> inv row
  - KV^T = WkvT.T @ xT;  K^T duplicated into both 64-partition halves (k2)
  - V^T -> PE-transpose -> V' = [V | ones]  (ones column => softmax denominator)
  - Q^T = (Wq*gamma*scale)T.T @ xT, * inv[i] broadcast on eviction
  - per (i-block 512, head-pair): S^T[j,i] 2 heads row-packed per PE pass ->
    exp on ScalarE (PSUM [128,1024] -> SBUF) ->
    U[65,512] += V'.T @ expS  (row 64 = denominator) ->
    out^T = U[0:64] * bcast(1/U[64])  -> y[i,c] = sum_p outT_p.T @ WoT_ls
All matmuls run as float32r (full PE rate for moving dim >= 256, ~tf32
precision); data stored fp32.
"""

import sys

sys.path.insert(0, "/opt/trn_rl_repo")

from contextlib import ExitStack

import numpy as np

import concourse.bass as bass
import concourse.tile as tile
from concourse import bacc, masks, mybir
from concourse.bass_utils import run_bass_kernel_spmd

FP = mybir.dt.float32
FR = mybir.dt.float32r
AF = mybir.ActivationFunctionType

B, N, C = 4, 2048, 1024
D = 64  # head dim
HCORE = 8  # query heads per core
PAIRS = HCORE // 2  # 4 head-pairs per core
CC = C // 128  # 8 contraction chunks
NB = N // 512  # 4 token blocks
JT = N // 128  # 16 key tiles
EPS = 1e-7
SCALE = D**-0.5
NCORES = 8


def _r(ap):
    """float32r view of an fp32 AP for full-rate PE matmuls."""
    return ap.bitcast(FR)


def _emit(tc, xT_d, wq_d, wkv_d, wo_d, y_d):
    nc = tc.nc

    with ExitStack() as top:
        consts = top.enter_context(tc.tile_pool(name="consts", bufs=1))
        wo_p = top.enter_context(tc.tile_pool(name="wo", bufs=PAIRS))
        qt_p = top.enter_context(tc.tile_pool(name="qt", bufs=PAIRS))
        k2_p = top.enter_context(tc.tile_pool(name="k2", bufs=1))
        vp_p = top.enter_context(tc.tile_pool(name="vp", bufs=JT))
        misc_p = top.enter_context(tc.tile_pool(name="miscsb", bufs=1))
        rows_p = top.enter_context(tc.tile_pool(name="rows", bufs=8))

        ones_f32 = consts.tile([128, 1], FP, tag="ones_f32")
        nc.vector.memset(ones_f32[:], 1.0)
        ones_col = consts.tile([128, 1], FR, tag="ones_col")
        nc.vector.tensor_copy(ones_col[:], ones_f32[:])
        identity = consts.tile([128, 128], FP, tag="ident")
        masks.make_identity(nc, identity[:])

        wo = []
        for p in range(PAIRS):
            t = wo_p.tile([128, C], FR, tag="wo")
            nc.sync.dma_start(t[:], wo_d[p * 128 : (p + 1) * 128, :].bitcast(FR))
            wo.append(t)

        inv_row = misc_p.tile([1, N], FP, tag="inv_row")
        inv_bc = misc_p.tile([128, N], FP, tag="inv_bc")
        k2 = k2_p.tile([128, N], FR, tag="k2")
        vt = misc_p.tile([64, N], FP, tag="vt")
        qt = [qt_p.tile([128, N], FR, tag="qt", name=f"qt{i}") for i in range(PAIRS)]
        vp = [vp_p.tile([128, D + 1], FR, tag="vp", name=f"vp{i}") for i in range(JT)]

        # ---------------- phase 1: LN stats + KV + Q projections ------------
        with (
            tc.tile_pool(name="wq", bufs=CC) as wq_p,
            tc.tile_pool(name="wkv", bufs=CC) as wkv_p,
            tc.tile_pool(name="xtb", bufs=3) as xtb_p,
            tc.tile_pool(name="xsq", bufs=2) as xsq_p,
            tc.tile_pool(name="pssm", bufs=1, space="PSUM") as pssm_p,
            tc.tile_pool(name="psbg", bufs=6, space="PSUM") as psbg_p,
        ):
            wq = []
            for c in range(CC):
                t = wq_p.tile([128, HCORE * D], FR, tag="wq")
                nc.sync.dma_start(t[:], wq_d[c * 128 : (c + 1) * 128, :].bitcast(FR))
                wq.append(t)
            wkv = []
            for c in range(CC):
                t = wkv_p.tile([128, 2 * D], FR, tag="wkv")
                nc.sync.dma_start(t[:], wkv_d[c * 128 : (c + 1) * 128, :].bitcast(FR))
                wkv.append(t)

            for nb in range(NB):
                sl = bass.ts(nb, 512)
                ps_s = pssm_p.tile([1, 512], FP, tag="ps_s")
                ps_q = pssm_p.tile([1, 512], FP, tag="ps_q")
                kv_ps = psbg_p.tile([128, 512], FP, tag="psbg")
                q_ps = [psbg_p.tile([128, 512], FP, tag="psbg", name=f"qps{i}") for i in range(PAIRS)]
                for c in range(CC):
                    xtc = xtb_p.tile([128, 512], FR, tag="xtb")
                    nc.sync.dma_start(
                        xtc[:], xT_d[c * 128 : (c + 1) * 128, sl].bitcast(FR)
                    )
                    st, sp = (c == 0), (c == CC - 1)
                    nc.tensor.matmul(
                        ps_s[:], ones_col[:], xtc[:], start=st, stop=sp
                    )
                    xq = xsq_p.tile([128, 512], FR, tag="xsq")
                    nc.vector.tensor_mul(
                        xq[:], xtc[:].bitcast(FP), xtc[:].bitcast(FP)
                    )
                    nc.tensor.matmul(
                        ps_q[:], ones_col[:], xq[:], start=st, stop=sp
                    )
                    nc.tensor.matmul(
                        kv_ps[:], wkv[c][:], xtc[:], start=st, stop=sp
                    )
                    for p in range(PAIRS):
                        nc.tensor.matmul(
                            q_ps[p][:],
                            wq[c][:, p * 128 : (p + 1) * 128],
                            xtc[:],
                            start=st,
                            stop=sp,
                        )
                # KV eviction: k duplicated into both halves; v^T staged
                nc.vector.tensor_copy(k2[0:64, sl], kv_ps[0:64, :])
                nc.vector.tensor_copy(k2[64:128, sl], kv_ps[0:64, :])
                nc.vector.tensor_copy(vt[:, sl], kv_ps[64:128, :])
                # LN: var = (ssq - sum^2/C)/(C-1); inv = 1/(sqrt(var)+eps)
                s_row = rows_p.tile([1, 512], FP, tag="row")
                nc.vector.tensor_copy(s_row[:], ps_s[:])
                nc.vector.tensor_mul(s_row[:], s_row[:], s_row[:])
                nc.vector.tensor_scalar_mul(s_row[:], s_row[:], 1.0 / C)
                v_row = rows_p.tile([1, 512], FP, tag="row")
                nc.vector.tensor_sub(v_row[:], ps_q[:], s_row[:])
                nc.vector.tensor_scalar_mul(v_row[:], v_row[:], 1.0 / (C - 1))
                std_row = rows_p.tile([1, 512], FP, tag="row")
                nc.scalar.activation(std_row[:], v_row[:], AF.Sqrt)
                nc.vector.tensor_scalar_add(std_row[:], std_row[:], EPS)
                nc.vector.reciprocal(inv_row[0:1, sl], std_row[:])
                # broadcast inv over 128 partitions (gpsimd)
                nc.gpsimd.partition_broadcast(inv_bc[:, sl], inv_row[0:1, sl])
                # Q eviction with 1/std applied
                for p in range(PAIRS):
                    nc.vector.tensor_mul(qt[p][:, sl], q_ps[p][:], inv_bc[:, sl])

        # ---------------- phase 1b: V natural + ones column ------------------
        with tc.tile_pool(name="pstr", bufs=2, space="PSUM") as pstr_p:
            for jt in range(JT):
                tr_ps = pstr_p.tile([128, D], FP, tag="pstr")
                nc.tensor.transpose(
                    tr_ps[:, 0:D],
                    vt[:, jt * 128 : (jt + 1) * 128],
                    identity[0:64, 0:64],
                )
                nc.vector.tensor_copy(vp[jt][:, 0:D], tr_ps[:, 0:D])
                nc.vector.tensor_copy(vp[jt][:, D : D + 1], ones_f32[:])

        # ---------------- phase 2: attention + out projection ----------------
        with (
            tc.tile_pool(name="pss", bufs=2, space="PSUM") as pss_p,
            tc.tile_pool(name="psu", bufs=2, space="PSUM") as psu_p,
            tc.tile_pool(name="psy", bufs=2, space="PSUM") as psy_p,
            tc.tile_pool(name="es", bufs=4) as es_p,
            tc.tile_pool(name="ot", bufs=2 * PAIRS) as ot_p,
            tc.tile_pool(name="bc", bufs=2) as bc_p,
            tc.tile_pool(name="ysb", bufs=2) as ysb_p,
        ):
            for ib in range(NB):
                isl = bass.ts(ib, 512)
                ots = []
                for p in range(PAIRS):
                    uA = psu_p.tile([D + 1, 512], FP, tag="u")
                    uB = psu_p.tile([D + 1, 512], FP, tag="u")
                    for jt in range(JT):
                        jsl = bass.ts(jt, 128)
                        s2 = pss_p.tile([128, 1024], FP, tag="s2")
                        # S^T for the two heads of the pair: row-packed
                        # (64-partition contractions in array rows 0-63/64-127)
                        nc.tensor.matmul(
                            s2[:, 0:512],
                            k2[0:64, jsl],
                            qt[p][0:64, isl],
                            start=True, stop=True,
                        )
                        nc.tensor.matmul(
                            s2[:, 512:1024],
                            k2[64:128, jsl],
                            qt[p][64:128, isl],
                            start=True, stop=True,
                        )
                        est = es_p.tile([128, 1024], FR, tag="es")
                        nc.scalar.activation(est[:], s2[:], AF.Exp)
                        nc.tensor.matmul(
                            uA[:], vp[jt][:], est[:, 0:512],
                            start=(jt == 0), stop=(jt == JT - 1),
                        )
                        nc.tensor.matmul(
                            uB[:], vp[jt][:], est[:, 512:1024],
                            start=(jt == 0), stop=(jt == JT - 1),
                        )
                    recA = rows_p.tile([1, 512], FP, tag="row")
                    nc.vector.reciprocal(recA[:], uA[D : D + 1, :])
                    recB = rows_p.tile([1, 512], FP, tag="row")
                    nc.vector.reciprocal(recB[:], uB[D : D + 1, :])
                    bcA = bc_p.tile([64, 512], FP, tag="bc")
                    nc.gpsimd.partition_broadcast(bcA[:], recA[:])
                    bcB = bc_p.tile([64, 512], FP, tag="bc")
                    nc.gpsimd.partition_broadcast(bcB[:], recB[:])
                    ot = ot_p.tile([128, 512], FR, tag="ot")
                    nc.vector.tensor_mul(ot[0:64, :], uA[0:64, :], bcA[:])
                    nc.vector.tensor_mul(ot[64:128, :], uB[0:64, :], bcB[:])
                    ots.append(ot)
                for t in range(4):
                    it = ib * 4 + t
                    tsl = bass.ds(t * 128, 128)
                    for cb in range(2):
                        csl = bass.ts(cb, 512)
                        y_ps = psy_p.tile([128, 512], FP, tag="y")
                        for p in range(PAIRS):
                            nc.tensor.matmul(
                                y_ps[:],
                                ots[p][:, tsl],
                                wo[p][:, csl],
                                start=(p == 0), stop=(p == PAIRS - 1),
                            )
                        y_sb = ysb_p.tile([128, 512], FP, tag="ysb")
                        nc.vector.tensor_copy(y_sb[:], y_ps[:])
                        nc.sync.dma_start(
                            y_d[it * 128 : (it + 1) * 128, csl], y_sb[:]
                        )


def build_program():
    nc = bacc.Bacc(
        "TRN2",
        target_bir_lowering=False,
        debug=False,
        enable_asserts=False,
        num_devices=NCORES,
    )
    xT_d = nc.dram_tensor("xT", [C, N], FP, kind="ExternalInput").ap()
    wq_d = nc.dram_tensor("wqT", [C, HCORE * D], FP, kind="ExternalInput").ap()
    wkv_d = nc.dram_tensor("wkvT", [C, 2 * D], FP, kind="ExternalInput").ap()
    wo_d = nc.dram_tensor("woT", [HCORE * D, C], FP, kind="ExternalInput").ap()
    y_d = nc.dram_tensor("y", [N, C], FP, kind="ExternalOutput").ap()
    with tile.TileContext(nc) as tc:
        _emit(tc, xT_d, wq_d, wkv_d, wo_d, y_d)
    nc.compile()
    return nc


_NC_CACHE = None


def _get_nc():
    global _NC_CACHE
    if _NC_CACHE is None:
        _NC_CACHE = build_program()
    return _NC_CACHE


def make_in_maps(x, gamma, Wq, Wkv, Wo, ls_scale):
    """Host-side sharding/layout prep (layout transforms + tiny weight folds)."""
    x = np.asarray(x, np.float32)
    gamma = np.asarray(gamma, np.float32).reshape(C)
    Wq = np.asarray(Wq, np.float32)
    Wkv = np.asarray(Wkv, np.float32)
    Wo = np.asarray(Wo, np.float32)
    ls = np.asarray(ls_scale, np.float32).reshape(C)

    wkvT = np.ascontiguousarray(Wkv.T)  # [C, 128]
    in_maps = []
    for core in range(NCORES):
        b, g = divmod(core, 2)
        hsl = slice(g * HCORE * D, (g + 1) * HCORE * D)
        wq_fold = Wq[hsl, :] * (gamma * SCALE)[None, :]  # [512, C]
        wo_fold = Wo[:, hsl] * ls[:, None]  # [C, 512]
        in_maps.append(
            {
                "xT": np.ascontiguousarray(x[b].T),
                "wqT": np.ascontiguousarray(wq_fold.T),
                "wkvT": wkvT,
                "woT": np.ascontiguousarray(wo_fold.T),
            }
        )
    return in_maps


def run_cores(in_maps, trace=False, **kw):
    nc = _get_nc()
    return run_bass_kernel_spmd(nc, in_maps, list(range(NCORES)), trace=trace, **kw)


def kernel(x, gamma, Wq, Wkv, Wo, ls_scale):
    in_maps = make_in_maps(x, gamma, Wq, Wkv, Wo, ls_scale)
    res = run_cores(in_maps)
    out = np.empty((B, N, C), np.float32)
    for b in range(B):
        out[b] = res.results[2 * b]["y"] + res.results[2 * b + 1]["y"]
    return out


if __name__ == "__main__":
    nc = _get_nc()
    print("program built:", nc)


# revision 7
# speedup vs baseline: 1.0832x; 1.0832x over previous
"""Trainium2 Bass kernel: BiasFreeLayerNorm + MQA attention + out-proj.

Problem (nn_Attention_90812788506696):
  x[B=4, N=2048, C=1024]; std over C (ddof=1, no mean subtraction of x);
  xn = x/(std+eps)*gamma; q = xn@Wq.T (16 heads x 64); k,v = x@Wkv.T (1 shared
  kv head, MQA); softmax(q k^T / sqrt(64)) @ v; concat; @Wo.T; * ls_scale.

Sharding (8 cores): core = (batch b = core//2, head-group g = core%2 of 8
query heads). K/V replicated per batch. Each core produces a PARTIAL
y_part[b] = attn_out(8 heads) @ Wo[:, g-slice].T (ls folded); host sums the
two partials per batch. No device collectives.

Device dataflow per core (feature-major layout; "T" = [features, tokens]):
  - stream xT slices; LN stats via ones-BLOCK matmul (sums land broadcast on
    all 128 PSUM partitions) -> full-width var/std -> inv_bc [128, n]
  - KV^T = WkvT.T @ xT;  K^T duplicated into both 64-partition halves (k2)
  - V^T -> PE-transpose -> V'' = [V | ones-block]  (cols 64:128 all-ones =>
    U rows 64:128 hold the softmax denominator pre-broadcast)
  - Q^T = (Wq*gamma*scale)T.T @ xT, * inv[i] on eviction
  - per (i-block 512, head-pair): S^T[j,i] 2 heads row-packed per PE pass ->
    exp on ScalarE (PSUM [128,1024] -> SBUF bf16) ->
    U[128,512] += V''.T @ expS  (rows 64:128 = denominator) ->
    out^T = U[0:64] * reciprocal(U[64:128]) -> y[i,c] = sum_p outT_p.T @ WoT
All matmul operands are bf16 (fp32 PSUM accumulation); softmax statistics,
normalization and divisions are fp32.
"""

import sys

sys.path.insert(0, "/opt/trn_rl_repo")

from contextlib import ExitStack

import ml_dtypes
import numpy as np

import concourse.bass as bass
import concourse.tile as tile
from concourse import bacc, masks, mybir
from concourse.bass_utils import run_bass_kernel_spmd

FP = mybir.dt.float32
BF = mybir.dt.bfloat16
AF = mybir.ActivationFunctionType
OP = mybir.AluOpType

B, N, C = 4, 2048, 1024
D = 64  # head dim
HCORE = 8  # query heads per core
PAIRS = HCORE // 2  # 4 head-pairs per core
CC = C // 128  # 8 contraction chunks
NB = N // 512  # 4 token blocks
JT = N // 128  # 16 key tiles
EPS = 1e-7
SCALE = D**-0.5
NCORES = 8


def _emit(tc, xT_d, wq_d, wkv_d, wo_d, y_d):
    nc = tc.nc

    with ExitStack() as top:
        consts = top.enter_context(tc.tile_pool(name="consts", bufs=1))
        wo_p = top.enter_context(tc.tile_pool(name="wo", bufs=PAIRS))
        qt_p = top.enter_context(tc.tile_pool(name="qt", bufs=PAIRS))
        k2_p = top.enter_context(tc.tile_pool(name="k2", bufs=1))
        vp_p = top.enter_context(tc.tile_pool(name="vp", bufs=JT))
        misc_p = top.enter_context(tc.tile_pool(name="miscsb", bufs=1))

        ones_f32 = consts.tile([128, 128], FP, tag="ones_f32")
        nc.vector.memset(ones_f32[:], 1.0)
        ones_blk = consts.tile([128, 128], BF, tag="ones_blk")
        nc.vector.tensor_copy(ones_blk[:], ones_f32[:])
        identity = consts.tile([128, 128], FP, tag="ident")
        masks.make_identity(nc, identity[:])

        wo = []
        for p in range(PAIRS):
            t = wo_p.tile([128, C], BF, tag="wo")
            nc.sync.dma_start(t[:], wo_d[p * 128 : (p + 1) * 128, :])
            wo.append(t)

        inv_bc = misc_p.tile([128, N], FP, tag="inv_bc")
        k2 = k2_p.tile([128, N], BF, tag="k2")
        vt = misc_p.tile([64, N], FP, tag="vt")
        qt = [qt_p.tile([128, N], BF, tag="qt", name=f"qt{i}") for i in range(PAIRS)]
        vp = [vp_p.tile([128, 128], BF, tag="vp", name=f"vp{i}") for i in range(JT)]

        # ---------------- phase 1: LN stats + KV + Q projections ------------
        with (
            tc.tile_pool(name="wq", bufs=CC) as wq_p,
            tc.tile_pool(name="wkv", bufs=CC) as wkv_p,
            tc.tile_pool(name="xtb", bufs=3) as xtb_p,
            tc.tile_pool(name="xsq", bufs=2) as xsq_p,
            tc.tile_pool(name="rows", bufs=4) as rows_p,
            tc.tile_pool(name="pssm", bufs=1, space="PSUM") as pssm_p,
            tc.tile_pool(name="psbg", bufs=6, space="PSUM") as psbg_p,
        ):
            wq = []
            for c in range(CC):
                t = wq_p.tile([128, HCORE * D], BF, tag="wq")
                nc.sync.dma_start(t[:], wq_d[c * 128 : (c + 1) * 128, :])
                wq.append(t)
            wkv = []
            for c in range(CC):
                t = wkv_p.tile([128, 2 * D], BF, tag="wkv")
                nc.sync.dma_start(t[:], wkv_d[c * 128 : (c + 1) * 128, :])
                wkv.append(t)

            for nb in range(NB):
                sl = bass.ts(nb, 512)
                ps_s = pssm_p.tile([128, 512], FP, tag="ps_s")
                ps_q = pssm_p.tile([128, 512], FP, tag="ps_q")
                kv_ps = psbg_p.tile([128, 512], FP, tag="psbg")
                q_ps = [
                    psbg_p.tile([128, 512], FP, tag="psbg", name=f"qps{i}")
                    for i in range(PAIRS)
                ]
                for c in range(CC):
                    xtc = xtb_p.tile([128, 512], BF, tag="xtb")
                    nc.sync.dma_start(
                        xtc[:], xT_d[c * 128 : (c + 1) * 128, sl]
                    )
                    st, sp = (c == 0), (c == CC - 1)
                    # sums land broadcast on all 128 partitions (ones block)
                    nc.tensor.matmul(
                        ps_s[:], ones_blk[:], xtc[:], start=st, stop=sp
                    )
                    xq = xsq_p.tile([128, 512], BF, tag="xsq")
                    nc.vector.tensor_mul(xq[:], xtc[:], xtc[:])
                    nc.tensor.matmul(
                        ps_q[:], ones_blk[:], xq[:], start=st, stop=sp
                    )
                    nc.tensor.matmul(
                        kv_ps[:], wkv[c][:], xtc[:], start=st, stop=sp
                    )
                    for p in range(PAIRS):
                        nc.tensor.matmul(
                            q_ps[p][:],
                            wq[c][:, p * 128 : (p + 1) * 128],
                            xtc[:],
                            start=st,
                            stop=sp,
                        )
                # KV eviction: k duplicated into both halves; v^T staged
                nc.vector.tensor_copy(k2[0:64, sl], kv_ps[0:64, :])
                nc.vector.tensor_copy(k2[64:128, sl], kv_ps[0:64, :])
                nc.vector.tensor_copy(vt[:, sl], kv_ps[64:128, :])
                # LN (full-width): var = (ssq - sum^2/C)/(C-1)
                s_sb = rows_p.tile([128, 512], FP, tag="row")
                nc.vector.tensor_copy(s_sb[:], ps_s[:])
                t1 = rows_p.tile([128, 512], FP, tag="row")
                nc.vector.tensor_mul(t1[:], s_sb[:], s_sb[:])
                # t2 = ps_q - t1/C
                t2 = rows_p.tile([128, 512], FP, tag="row")
                nc.vector.scalar_tensor_tensor(
                    t2[:], t1[:], -1.0 / C, ps_q[:], OP.mult, OP.add
                )
                # std = sqrt(t2/(C-1)); inv_bc = 1/(std+eps)
                std_sb = rows_p.tile([128, 512], FP, tag="row")
                nc.scalar.activation(
                    std_sb[:], t2[:], AF.Sqrt, scale=1.0 / (C - 1)
                )
                nc.vector.tensor_scalar_add(std_sb[:], std_sb[:], EPS)
                nc.vector.reciprocal(inv_bc[:, sl], std_sb[:])
                # Q eviction with 1/std applied (bf16 out)
                for p in range(PAIRS):
                    nc.vector.tensor_mul(qt[p][:, sl], q_ps[p][:], inv_bc[:, sl])

        # ---------------- phase 1b: V natural + ones block -------------------
        with tc.tile_pool(name="pstr", bufs=2, space="PSUM") as pstr_p:
            for jt in range(JT):
                tr_ps = pstr_p.tile([128, D], FP, tag="pstr")
                nc.tensor.transpose(
                    tr_ps[:, 0:D],
                    vt[:, jt * 128 : (jt + 1) * 128],
                    identity[0:64, 0:64],
                )
                nc.vector.tensor_copy(vp[jt][:, 0:D], tr_ps[:, 0:D])
                nc.vector.tensor_copy(vp[jt][:, D:128], ones_f32[:, 0:D])

        # ---------------- phase 2: attention + out projection ----------------
        with (
            tc.tile_pool(name="pss", bufs=2, space="PSUM") as pss_p,
            tc.tile_pool(name="psu", bufs=3, space="PSUM") as psu_p,
            tc.tile_pool(name="psy", bufs=1, space="PSUM") as psy_p,
            tc.tile_pool(name="es", bufs=6) as es_p,
            tc.tile_pool(name="ot", bufs=2 * PAIRS) as ot_p,
            tc.tile_pool(name="rec", bufs=4) as rec_p,
            tc.tile_pool(name="ysb", bufs=2) as ysb_p,
        ):
            for ib in range(NB):
                isl = bass.ts(ib, 512)
                ots = []
                for p in range(PAIRS):
                    uA = psu_p.tile([128, 512], FP, tag="u")
                    uB = psu_p.tile([128, 512], FP, tag="u")
                    for jt in range(JT):
                        jsl = bass.ts(jt, 128)
                        s2 = pss_p.tile([128, 1024], FP, tag="s2")
                        # S^T for the two heads of the pair: row-packed
                        # (64-part contractions in array rows 0-63/64-127)
                        nc.tensor.matmul(
                            s2[:, 0:512],
                            k2[0:64, jsl],
                            qt[p][0:64, isl],
                            start=True, stop=True,
                        )
                        nc.tensor.matmul(
                            s2[:, 512:1024],
                            k2[64:128, jsl],
                            qt[p][64:128, isl],
                            start=True, stop=True,
                        )
                        est = es_p.tile([128, 1024], BF, tag="es")
                        nc.scalar.activation(est[:], s2[:], AF.Exp)
                        nc.tensor.matmul(
                            uA[:], vp[jt][:], est[:, 0:512],
                            start=(jt == 0), stop=(jt == JT - 1),
                        )
                        nc.tensor.matmul(
                            uB[:], vp[jt][:], est[:, 512:1024],
                            start=(jt == 0), stop=(jt == JT - 1),
                        )
                    # softmax division: denom sits broadcast in U rows 64:128
                    recA = rec_p.tile([64, 512], FP, tag="rec")
                    nc.vector.reciprocal(recA[:], uA[64:128, :])
                    recB = rec_p.tile([64, 512], FP, tag="rec")
                    nc.vector.reciprocal(recB[:], uB[64:128, :])
                    ot = ot_p.tile([128, 512], BF, tag="ot")
                    nc.vector.tensor_mul(ot[0:64, :], uA[0:64, :], recA[:])
                    nc.vector.tensor_mul(ot[64:128, :], uB[0:64, :], recB[:])
                    ots.append(ot)
                for t in range(4):
                    it = ib * 4 + t
                    tsl = bass.ds(t * 128, 128)
                    for cb in range(2):
                        csl = bass.ts(cb, 512)
                        y_ps = psy_p.tile([128, 512], FP, tag="y")
                        for p in range(PAIRS):
                            nc.tensor.matmul(
                                y_ps[:],
                                ots[p][:, tsl],
                                wo[p][:, csl],
                                start=(p == 0), stop=(p == PAIRS - 1),
                            )
                        y_sb = ysb_p.tile([128, 512], FP, tag="ysb")
                        nc.vector.tensor_copy(y_sb[:], y_ps[:])
                        nc.sync.dma_start(
                            y_d[it * 128 : (it + 1) * 128, csl], y_sb[:]
                        )


def build_program():
    nc = bacc.Bacc(
        "TRN2",
        target_bir_lowering=False,
        debug=False,
        enable_asserts=False,
        num_devices=NCORES,
    )
    xT_d = nc.dram_tensor("xT", [C, N], BF, kind="ExternalInput").ap()
    wq_d = nc.dram_tensor("wqT", [C, HCORE * D], BF, kind="ExternalInput").ap()
    wkv_d = nc.dram_tensor("wkvT", [C, 2 * D], BF, kind="ExternalInput").ap()
    wo_d = nc.dram_tensor("woT", [HCORE * D, C], BF, kind="ExternalInput").ap()
    y_d = nc.dram_tensor("y", [N, C], FP, kind="ExternalOutput").ap()
    with tile.TileContext(nc) as tc:
        _emit(tc, xT_d, wq_d, wkv_d, wo_d, y_d)
    nc.compile()
    return nc


_NC_CACHE = None


def _get_nc():
    global _NC_CACHE
    if _NC_CACHE is None:
        _NC_CACHE = build_program()
    return _NC_CACHE


def make_in_maps(x, gamma, Wq, Wkv, Wo, ls_scale):
    """Host-side sharding/layout prep (layout transforms + tiny weight folds)."""
    bf16 = ml_dtypes.bfloat16
    x = np.asarray(x, np.float32)
    gamma = np.asarray(gamma, np.float32).reshape(C)
    Wq = np.asarray(Wq, np.float32)
    Wkv = np.asarray(Wkv, np.float32)
    Wo = np.asarray(Wo, np.float32)
    ls = np.asarray(ls_scale, np.float32).reshape(C)

    wkvT = np.ascontiguousarray(Wkv.T).astype(bf16)  # [C, 128]
    in_maps = []
    for core in range(NCORES):
        b, g = divmod(core, 2)
        hsl = slice(g * HCORE * D, (g + 1) * HCORE * D)
        wq_fold = Wq[hsl, :] * (gamma * SCALE)[None, :]  # [512, C]
        wo_fold = Wo[:, hsl] * ls[:, None]  # [C, 512]
        in_maps.append(
            {
                "xT": np.ascontiguousarray(x[b].T).astype(bf16),
                "wqT": np.ascontiguousarray(wq_fold.T).astype(bf16),
                "wkvT": wkvT,
                "woT": np.ascontiguousarray(wo_fold.T).astype(bf16),
            }
        )
    return in_maps


def run_cores(in_maps, trace=False, **kw):
    nc = _get_nc()
    return run_bass_kernel_spmd(nc, in_maps, list(range(NCORES)), trace=trace, **kw)


def kernel(x, gamma, Wq, Wkv, Wo, ls_scale):
    in_maps = make_in_maps(x, gamma, Wq, Wkv, Wo, ls_scale)
    res = run_cores(in_maps)
    out = np.empty((B, N, C), np.float32)
    for b in range(B):
        out[b] = res.results[2 * b]["y"] + res.results[2 * b + 1]["y"]
    return out


if __name__ == "__main__":
    nc = _get_nc()
    print("program built:", nc)


# revision 11
# speedup vs baseline: 1.1288x; 1.0421x over previous
"""Trainium2 Bass kernel: BiasFreeLayerNorm + MQA attention + out-proj.

Problem (nn_Attention_90812788506696):
  x[B=4, N=2048, C=1024]; std over C (ddof=1, no mean subtraction of x);
  xn = x/(std+eps)*gamma; q = xn@Wq.T (16 heads x 64); k,v = x@Wkv.T (1 shared
  kv head, MQA); softmax(q k^T / sqrt(64)) @ v; concat; @Wo.T; * ls_scale.

Sharding (8 cores): core = (batch b = core//2, head-group g = core%2 of 8
query heads). K/V replicated per batch. Each core produces a PARTIAL
y_part[b] = attn_out(8 heads) @ Wo[:, g-slice].T (ls folded); host sums the
two partials per batch. No device collectives.

Device dataflow per core (feature-major layout; "T" = [features, tokens]):
  - stream xT slices; LN stats via ones-BLOCK matmul (sums land broadcast on
    all 128 PSUM partitions) -> full-width var/std -> inv_bc [128, n]
  - KV^T = WkvT.T @ xT;  K^T duplicated into both 64-partition halves (k2)
  - V^T -> PE-transpose -> V'' = [V | ones-block]  (cols 64:128 all-ones =>
    U rows 64:128 hold the softmax denominator pre-broadcast)
  - Q^T = (Wq*gamma*scale)T.T @ xT, * inv[i] on eviction
  - per (i-block 512, head-pair): S^T[j,i] 2 heads row-packed per PE pass ->
    exp on ScalarE (PSUM [128,1024] -> SBUF bf16) ->
    U[128,512] += V''.T @ expS  (rows 64:128 = denominator) ->
    out^T = U[0:64] * reciprocal(U[64:128]) -> y[i,c] = sum_p outT_p.T @ WoT
All matmul operands are bf16 (fp32 PSUM accumulation); softmax statistics,
normalization and divisions are fp32.
"""

import sys

sys.path.insert(0, "/opt/trn_rl_repo")

from contextlib import ExitStack

import ml_dtypes
import numpy as np

import concourse.bass as bass
import concourse.tile as tile
from concourse import bacc, masks, mybir
from concourse.bass_utils import run_bass_kernel_spmd

FP = mybir.dt.float32
BF = mybir.dt.bfloat16
AF = mybir.ActivationFunctionType
OP = mybir.AluOpType

B, N, C = 4, 2048, 1024
D = 64  # head dim
HCORE = 8  # query heads per core
PAIRS = HCORE // 2  # 4 head-pairs per core
CC = C // 128  # 8 contraction chunks
NB = N // 512  # 4 token blocks
JT = N // 128  # 16 key tiles
EPS = 1e-7
SCALE = D**-0.5
NCORES = 8


def _emit(tc, xT_d, wq_d, wkv_d, wo_d, y_d):
    nc = tc.nc

    with ExitStack() as top:
        consts = top.enter_context(tc.tile_pool(name="consts", bufs=1))
        wo_p = top.enter_context(tc.tile_pool(name="wo", bufs=PAIRS))
        qt_p = top.enter_context(tc.tile_pool(name="qt", bufs=PAIRS))
        k2_p = top.enter_context(tc.tile_pool(name="k2", bufs=1))
        vp_p = top.enter_context(tc.tile_pool(name="vp", bufs=JT))
        misc_p = top.enter_context(tc.tile_pool(name="miscsb", bufs=1))

        ones_f32 = consts.tile([128, 128], FP, tag="ones_f32")
        nc.vector.memset(ones_f32[:], 1.0)
        ones_blk = consts.tile([128, 128], BF, tag="ones_blk")
        nc.vector.tensor_copy(ones_blk[:], ones_f32[:])

        wo = []
        for p in range(PAIRS):
            t = wo_p.tile([128, C], BF, tag="wo")
            nc.sync.dma_start(t[:], wo_d[p * 128 : (p + 1) * 128, :])
            wo.append(t)

        inv_bc = misc_p.tile([128, N], FP, tag="inv_bc")
        k2 = k2_p.tile([128, N], BF, tag="k2")
        vt = misc_p.tile([64, N], BF, tag="vt")
        qt = [qt_p.tile([128, N], BF, tag="qt", name=f"qt{i}") for i in range(PAIRS)]
        vp = [vp_p.tile([128, 128], BF, tag="vp", name=f"vp{i}") for i in range(JT)]

        # ---------------- phase 1: LN stats + KV + Q projections ------------
        with (
            tc.tile_pool(name="wq", bufs=CC) as wq_p,
            tc.tile_pool(name="wkv", bufs=CC) as wkv_p,
            tc.tile_pool(name="xtb", bufs=3) as xtb_p,
            tc.tile_pool(name="xsq", bufs=2) as xsq_p,
            tc.tile_pool(name="rows", bufs=4) as rows_p,
            tc.tile_pool(name="pssm", bufs=1, space="PSUM") as pssm_p,
            tc.tile_pool(name="psbg", bufs=6, space="PSUM") as psbg_p,
        ):
            wq = []
            for c in range(CC):
                t = wq_p.tile([128, HCORE * D], BF, tag="wq")
                nc.sync.dma_start(t[:], wq_d[c * 128 : (c + 1) * 128, :])
                wq.append(t)
            wkv = []
            for c in range(CC):
                t = wkv_p.tile([128, 2 * D], BF, tag="wkv")
                nc.sync.dma_start(t[:], wkv_d[c * 128 : (c + 1) * 128, :])
                wkv.append(t)

            for nb in range(NB):
                sl = bass.ts(nb, 512)
                ps_s = pssm_p.tile([128, 512], FP, tag="ps_s")
                ps_q = pssm_p.tile([128, 512], FP, tag="ps_q")
                kv_ps = psbg_p.tile([128, 512], FP, tag="psbg")
                q_ps = [
                    psbg_p.tile([128, 512], FP, tag="psbg", name=f"qps{i}")
                    for i in range(PAIRS)
                ]
                for c in range(CC):
                    xtc = xtb_p.tile([128, 512], BF, tag="xtb")
                    nc.sync.dma_start(
                        xtc[:], xT_d[c * 128 : (c + 1) * 128, sl]
                    )
                    st, sp = (c == 0), (c == CC - 1)
                    # sums land broadcast on all 128 partitions (ones block)
                    nc.tensor.matmul(
                        ps_s[:], ones_blk[:], xtc[:], start=st, stop=sp
                    )
                    xq = xsq_p.tile([128, 512], BF, tag="xsq")
                    nc.vector.tensor_mul(xq[:], xtc[:], xtc[:])
                    nc.tensor.matmul(
                        ps_q[:], ones_blk[:], xq[:], start=st, stop=sp
                    )
                    nc.tensor.matmul(
                        kv_ps[:], wkv[c][:], xtc[:], start=st, stop=sp
                    )
                    for p in range(PAIRS):
                        nc.tensor.matmul(
                            q_ps[p][:],
                            wq[c][:, p * 128 : (p + 1) * 128],
                            xtc[:],
                            start=st,
                            stop=sp,
                        )
                # KV eviction: k duplicated into both halves; v^T staged
                nc.vector.tensor_copy(k2[0:64, sl], kv_ps[0:64, :])
                nc.vector.tensor_copy(k2[64:128, sl], kv_ps[0:64, :])
                nc.vector.tensor_copy(vt[:, sl], kv_ps[64:128, :])
                # LN (full-width): var = (ssq - sum^2/C)/(C-1)
                s_sb = rows_p.tile([128, 512], FP, tag="row")
                nc.vector.tensor_copy(s_sb[:], ps_s[:])
                t1 = rows_p.tile([128, 512], FP, tag="row")
                nc.vector.tensor_mul(t1[:], s_sb[:], s_sb[:])
                # t2 = ps_q - t1/C
                t2 = rows_p.tile([128, 512], FP, tag="row")
                nc.vector.scalar_tensor_tensor(
                    t2[:], t1[:], -1.0 / C, ps_q[:], OP.mult, OP.add
                )
                # std = sqrt(t2/(C-1)); inv_bc = 1/(std+eps)
                std_sb = rows_p.tile([128, 512], FP, tag="row")
                nc.scalar.activation(
                    std_sb[:], t2[:], AF.Sqrt, scale=1.0 / (C - 1)
                )
                nc.vector.tensor_scalar_add(std_sb[:], std_sb[:], EPS)
                nc.vector.reciprocal(inv_bc[:, sl], std_sb[:])
                # Q eviction with 1/std applied (bf16 out)
                for p in range(PAIRS):
                    nc.vector.tensor_mul(qt[p][:, sl], q_ps[p][:], inv_bc[:, sl])

        # ---------------- phase 1b: V natural + ones block -------------------
        # DMA xbar transpose (bf16): V^T [64,128] -> V [128,64], off the PE
        for jt in range(JT):
            nc.sync.dma_start_transpose(
                vp[jt][:, 0:D], vt[:, jt * 128 : (jt + 1) * 128]
            )
            nc.vector.tensor_copy(vp[jt][:, D:128], ones_f32[:, 0:D])

        # ---------------- phase 2: attention + out projection ----------------
        with (
            tc.tile_pool(name="pss", bufs=2, space="PSUM") as pss_p,
            tc.tile_pool(name="psu", bufs=3, space="PSUM") as psu_p,
            tc.tile_pool(name="psy", bufs=1, space="PSUM") as psy_p,
            tc.tile_pool(name="es", bufs=6) as es_p,
            tc.tile_pool(name="ot", bufs=2 * PAIRS) as ot_p,
            tc.tile_pool(name="rec", bufs=4) as rec_p,
            tc.tile_pool(name="ysb", bufs=2) as ysb_p,
        ):
            for ib in range(NB):
                isl = bass.ts(ib, 512)
                ots = []
                for p in range(PAIRS):
                    uA = psu_p.tile([128, 512], FP, tag="u")
                    uB = psu_p.tile([128, 512], FP, tag="u")
                    for jt in range(JT):
                        jsl = bass.ts(jt, 128)
                        s2 = pss_p.tile([128, 1024], FP, tag="s2")
                        # S^T for the two heads of the pair: row-packed
                        # (64-part contractions in array rows 0-63/64-127)
                        nc.tensor.matmul(
                            s2[:, 0:512],
                            k2[0:64, jsl],
                            qt[p][0:64, isl],
                            start=True, stop=True,
                        )
                        nc.tensor.matmul(
                            s2[:, 512:1024],
                            k2[64:128, jsl],
                            qt[p][64:128, isl],
                            start=True, stop=True,
                        )
                        est = es_p.tile([128, 1024], BF, tag="es")
                        nc.scalar.activation(est[:], s2[:], AF.Exp)
                        nc.tensor.matmul(
                            uA[:], vp[jt][:], est[:, 0:512],
                            start=(jt == 0), stop=(jt == JT - 1),
                        )
                        nc.tensor.matmul(
                            uB[:], vp[jt][:], est[:, 512:1024],
                            start=(jt == 0), stop=(jt == JT - 1),
                        )
                    # softmax division: denom sits broadcast in U rows 64:128
                    # 1/den = exp(-ln(den)) on ScalarE (ln+exp share one
                    # ACT table set; keeps the slow DVE reciprocal off the
                    # critical path)
                    lnA = rec_p.tile([64, 512], FP, tag="ln")
                    nc.scalar.activation(lnA[:], uA[64:128, :], AF.Ln)
                    recA = rec_p.tile([64, 512], FP, tag="rec")
                    nc.scalar.activation(recA[:], lnA[:], AF.Exp, scale=-1.0)
                    lnB = rec_p.tile([64, 512], FP, tag="ln")
                    nc.scalar.activation(lnB[:], uB[64:128, :], AF.Ln)
                    recB = rec_p.tile([64, 512], FP, tag="rec")
                    nc.scalar.activation(recB[:], lnB[:], AF.Exp, scale=-1.0)
                    ot = ot_p.tile([128, 512], BF, tag="ot")
                    nc.vector.tensor_mul(ot[0:64, :], uA[0:64, :], recA[:])
                    nc.vector.tensor_mul(ot[64:128, :], uB[0:64, :], recB[:])
                    ots.append(ot)
                for t in range(4):
                    it = ib * 4 + t
                    tsl = bass.ds(t * 128, 128)
                    for cb in range(2):
                        csl = bass.ts(cb, 512)
                        y_ps = psy_p.tile([128, 512], FP, tag="y")
                        for p in range(PAIRS):
                            nc.tensor.matmul(
                                y_ps[:],
                                ots[p][:, tsl],
                                wo[p][:, csl],
                                start=(p == 0), stop=(p == PAIRS - 1),
                            )
                        y_sb = ysb_p.tile([128, 512], FP, tag="ysb")
                        nc.vector.tensor_copy(y_sb[:], y_ps[:])
                        nc.sync.dma_start(
                            y_d[it * 128 : (it + 1) * 128, csl], y_sb[:]
                        )


def build_program():
    nc = bacc.Bacc(
        "TRN2",
        target_bir_lowering=False,
        debug=False,
        enable_asserts=False,
        num_devices=NCORES,
    )
    xT_d = nc.dram_tensor("xT", [C, N], BF, kind="ExternalInput").ap()
    wq_d = nc.dram_tensor("wqT", [C, HCORE * D], BF, kind="ExternalInput").ap()
    wkv_d = nc.dram_tensor("wkvT", [C, 2 * D], BF, kind="ExternalInput").ap()
    wo_d = nc.dram_tensor("woT", [HCORE * D, C], BF, kind="ExternalInput").ap()
    y_d = nc.dram_tensor("y", [N, C], FP, kind="ExternalOutput").ap()
    with tile.TileContext(nc) as tc:
        _emit(tc, xT_d, wq_d, wkv_d, wo_d, y_d)
    nc.compile()
    return nc


_NC_CACHE = None


def _get_nc():
    global _NC_CACHE
    if _NC_CACHE is None:
        _NC_CACHE = build_program()
    return _NC_CACHE


def make_in_maps(x, gamma, Wq, Wkv, Wo, ls_scale):
    """Host-side sharding/layout prep (layout transforms + tiny weight folds)."""
    bf16 = ml_dtypes.bfloat16
    x = np.asarray(x, np.float32)
    gamma = np.asarray(gamma, np.float32).reshape(C)
    Wq = np.asarray(Wq, np.float32)
    Wkv = np.asarray(Wkv, np.float32)
    Wo = np.asarray(Wo, np.float32)
    ls = np.asarray(ls_scale, np.float32).reshape(C)

    wkvT = np.ascontiguousarray(Wkv.T).astype(bf16)  # [C, 128]
    in_maps = []
    for core in range(NCORES):
        b, g = divmod(core, 2)
        hsl = slice(g * HCORE * D, (g + 1) * HCORE * D)
        wq_fold = Wq[hsl, :] * (gamma * SCALE)[None, :]  # [512, C]
        wo_fold = Wo[:, hsl] * ls[:, None]  # [C, 512]
        in_maps.append(
            {
                "xT": np.ascontiguousarray(x[b].T).astype(bf16),
                "wqT": np.ascontiguousarray(wq_fold.T).astype(bf16),
                "wkvT": wkvT,
                "woT": np.ascontiguousarray(wo_fold.T).astype(bf16),
            }
        )
    return in_maps


def run_cores(in_maps, trace=False, **kw):
    nc = _get_nc()
    return run_bass_kernel_spmd(nc, in_maps, list(range(NCORES)), trace=trace, **kw)


def kernel(x, gamma, Wq, Wkv, Wo, ls_scale):
    in_maps = make_in_maps(x, gamma, Wq, Wkv, Wo, ls_scale)
    res = run_cores(in_maps)
    out = np.empty((B, N, C), np.float32)
    for b in range(B):
        out[b] = res.results[2 * b]["y"] + res.results[2 * b + 1]["y"]
    return out


if __name__ == "__main__":
    nc = _get_nc()
    print("program built:", nc)


# revision 12
# speedup vs baseline: 1.2467x; 1.1044x over previous
"""Trainium2 Bass kernel: BiasFreeLayerNorm + MQA attention + out-proj.

Problem (nn_Attention_90812788506696):
  x[B=4, N=2048, C=1024]; std over C (ddof=1, no mean subtraction of x);
  xn = x/(std+eps)*gamma; q = xn@Wq.T (16 heads x 64); k,v = x@Wkv.T (1 shared
  kv head, MQA); softmax(q k^T / sqrt(64)) @ v; concat; @Wo.T; * ls_scale.

Sharding (8 cores): core = (batch b = core//2, head-group g = core%2 of 8
query heads). K/V replicated per batch. Each core produces a PARTIAL
y_part[b] = attn_out(8 heads) @ Wo[:, g-slice].T (ls folded); host sums the
two partials per batch. No device collectives.

Device dataflow per core (feature-major layout; "T" = [features, tokens]):
  - stream xT slices; LN stats via ones-BLOCK matmul (sums land broadcast on
    all 128 PSUM partitions) -> full-width var/std -> inv_bc [128, n]
  - KV^T = WkvT.T @ xT;  K^T duplicated into both 64-partition halves (k2)
  - V^T -> PE-transpose -> V'' = [V | ones-block]  (cols 64:128 all-ones =>
    U rows 64:128 hold the softmax denominator pre-broadcast)
  - Q^T = (Wq*gamma*scale)T.T @ xT, * inv[i] on eviction
  - per (i-block 512, head-pair): S^T[j,i] 2 heads row-packed per PE pass ->
    exp on ScalarE (PSUM [128,1024] -> SBUF bf16) ->
    U[128,512] += V''.T @ expS  (rows 64:128 = denominator) ->
    out^T = U[0:64] * reciprocal(U[64:128]) -> y[i,c] = sum_p outT_p.T @ WoT
All matmul operands are bf16 (fp32 PSUM accumulation); softmax statistics,
normalization and divisions are fp32.
"""

import sys

sys.path.insert(0, "/opt/trn_rl_repo")

from contextlib import ExitStack

import ml_dtypes
import numpy as np

import concourse.bass as bass
import concourse.tile as tile
from concourse import bacc, masks, mybir
from concourse.bass_utils import run_bass_kernel_spmd

FP = mybir.dt.float32
BF = mybir.dt.bfloat16
AF = mybir.ActivationFunctionType
OP = mybir.AluOpType

B, N, C = 4, 2048, 1024
D = 64  # head dim
HCORE = 8  # query heads per core
PAIRS = HCORE // 2  # 4 head-pairs per core
CC = C // 128  # 8 contraction chunks
NB = N // 512  # 4 token blocks
JT = N // 128  # 16 key tiles
EPS = 1e-7
SCALE = D**-0.5
NCORES = 8


def _emit(tc, xT_d, wq_d, wkv_d, wo_d, y_d):
    nc = tc.nc

    with ExitStack() as top:
        consts = top.enter_context(tc.tile_pool(name="consts", bufs=1))
        wo_p = top.enter_context(tc.tile_pool(name="wo", bufs=PAIRS))
        qt_p = top.enter_context(tc.tile_pool(name="qt", bufs=PAIRS))
        k2_p = top.enter_context(tc.tile_pool(name="k2", bufs=1))
        vp_p = top.enter_context(tc.tile_pool(name="vp", bufs=JT))
        misc_p = top.enter_context(tc.tile_pool(name="miscsb", bufs=1))

        ones_f32 = consts.tile([128, 128], FP, tag="ones_f32")
        nc.vector.memset(ones_f32[:], 1.0)
        ones_blk = consts.tile([128, 128], BF, tag="ones_blk")
        nc.vector.tensor_copy(ones_blk[:], ones_f32[:])

        wo = []
        for p in range(PAIRS):
            t = wo_p.tile([128, C], BF, tag="wo")
            nc.sync.dma_start(t[:], wo_d[p * 128 : (p + 1) * 128, :])
            wo.append(t)

        inv_bc = misc_p.tile([128, N], FP, tag="inv_bc")
        k2 = k2_p.tile([128, N], BF, tag="k2")
        vt = misc_p.tile([64, N], BF, tag="vt")
        qt = [qt_p.tile([128, N], BF, tag="qt", name=f"qt{i}") for i in range(PAIRS)]
        vp = [vp_p.tile([128, 128], BF, tag="vp", name=f"vp{i}") for i in range(JT)]

        # ---------------- phase 1: LN stats + KV + Q projections ------------
        with (
            tc.tile_pool(name="wq", bufs=CC) as wq_p,
            tc.tile_pool(name="wkv", bufs=CC) as wkv_p,
            tc.tile_pool(name="xtb", bufs=3) as xtb_p,
            tc.tile_pool(name="xsq", bufs=2) as xsq_p,
            tc.tile_pool(name="rows", bufs=4) as rows_p,
            tc.tile_pool(name="pssm", bufs=1, space="PSUM") as pssm_p,
            tc.tile_pool(name="psbg", bufs=6, space="PSUM") as psbg_p,
        ):
            wq = []
            for c in range(CC):
                t = wq_p.tile([128, HCORE * D], BF, tag="wq")
                nc.sync.dma_start(t[:], wq_d[c * 128 : (c + 1) * 128, :])
                wq.append(t)
            wkv = []
            for c in range(CC):
                t = wkv_p.tile([128, 2 * D], BF, tag="wkv")
                nc.sync.dma_start(t[:], wkv_d[c * 128 : (c + 1) * 128, :])
                wkv.append(t)

            for nb in range(NB):
                sl = bass.ts(nb, 512)
                ps_s = pssm_p.tile([128, 512], FP, tag="ps_s")
                ps_q = pssm_p.tile([128, 512], FP, tag="ps_q")
                kv_ps = psbg_p.tile([128, 512], FP, tag="psbg")
                q_ps = [
                    psbg_p.tile([128, 512], FP, tag="psbg", name=f"qps{i}")
                    for i in range(PAIRS)
                ]
                for c in range(CC):
                    xtc = xtb_p.tile([128, 512], BF, tag="xtb")
                    nc.sync.dma_start(
                        xtc[:], xT_d[c * 128 : (c + 1) * 128, sl]
                    )
                    st, sp = (c == 0), (c == CC - 1)
                    # sums land broadcast on all 128 partitions (ones block)
                    nc.tensor.matmul(
                        ps_s[:], ones_blk[:], xtc[:], start=st, stop=sp
                    )
                    xq = xsq_p.tile([128, 512], BF, tag="xsq")
                    nc.vector.tensor_mul(xq[:], xtc[:], xtc[:])
                    nc.tensor.matmul(
                        ps_q[:], ones_blk[:], xq[:], start=st, stop=sp
                    )
                    nc.tensor.matmul(
                        kv_ps[:], wkv[c][:], xtc[:], start=st, stop=sp
                    )
                    for p in range(PAIRS):
                        nc.tensor.matmul(
                            q_ps[p][:],
                            wq[c][:, p * 128 : (p + 1) * 128],
                            xtc[:],
                            start=st,
                            stop=sp,
                        )
                # KV eviction: k duplicated into both halves; v^T staged
                nc.vector.tensor_copy(k2[0:64, sl], kv_ps[0:64, :])
                nc.vector.tensor_copy(k2[64:128, sl], kv_ps[0:64, :])
                nc.vector.tensor_copy(vt[:, sl], kv_ps[64:128, :])
                # LN (full-width): var = (ssq - sum^2/C)/(C-1)
                s_sb = rows_p.tile([128, 512], FP, tag="row")
                nc.vector.tensor_copy(s_sb[:], ps_s[:])
                t1 = rows_p.tile([128, 512], FP, tag="row")
                nc.vector.tensor_mul(t1[:], s_sb[:], s_sb[:])
                # t2 = ps_q - t1/C
                t2 = rows_p.tile([128, 512], FP, tag="row")
                nc.vector.scalar_tensor_tensor(
                    t2[:], t1[:], -1.0 / C, ps_q[:], OP.mult, OP.add
                )
                # std = sqrt(t2/(C-1)); inv_bc = 1/(std+eps)
                std_sb = rows_p.tile([128, 512], FP, tag="row")
                nc.scalar.activation(
                    std_sb[:], t2[:], AF.Sqrt, scale=1.0 / (C - 1)
                )
                nc.vector.tensor_scalar_add(std_sb[:], std_sb[:], EPS)
                nc.vector.reciprocal(inv_bc[:, sl], std_sb[:])
                # Q eviction with 1/std applied (bf16 out)
                for p in range(PAIRS):
                    nc.vector.tensor_mul(qt[p][:, sl], q_ps[p][:], inv_bc[:, sl])

        # ---------------- phase 1b: V natural + ones block -------------------
        # DMA xbar transpose (bf16): V^T [64,128] -> V [128,64], off the PE
        for jt in range(JT):
            nc.sync.dma_start_transpose(
                vp[jt][:, 0:D], vt[:, jt * 128 : (jt + 1) * 128]
            )
            nc.vector.tensor_copy(vp[jt][:, D:128], ones_f32[:, 0:D])

        # ---------------- phase 2: attention + out projection ----------------
        with (
            tc.tile_pool(name="pss", bufs=2, space="PSUM") as pss_p,
            tc.tile_pool(name="psu", bufs=4, space="PSUM") as psu_p,
            tc.tile_pool(name="es", bufs=6) as es_p,
            tc.tile_pool(name="ot", bufs=2 * PAIRS) as ot_p,
            tc.tile_pool(name="rec", bufs=4) as rec_p,
            tc.tile_pool(name="ysb", bufs=2) as ysb_p,
        ):
            for ib in range(NB):
                isl = bass.ts(ib, 512)
                ots = []
                for p in range(PAIRS):
                    uA = psu_p.tile([128, 512], FP, tag="u")
                    uB = psu_p.tile([128, 512], FP, tag="u")
                    for jt in range(JT):
                        jsl = bass.ts(jt, 128)
                        s2 = pss_p.tile([128, 1024], FP, tag="s2")
                        # S^T for the two heads of the pair: row-packed
                        # (64-part contractions in array rows 0-63/64-127)
                        nc.tensor.matmul(
                            s2[:, 0:512],
                            k2[0:64, jsl],
                            qt[p][0:64, isl],
                            start=True, stop=True,
                        )
                        nc.tensor.matmul(
                            s2[:, 512:1024],
                            k2[64:128, jsl],
                            qt[p][64:128, isl],
                            start=True, stop=True,
                        )
                        est = es_p.tile([128, 1024], BF, tag="es")
                        nc.scalar.activation(est[:], s2[:], AF.Exp)
                        nc.tensor.matmul(
                            uA[:], vp[jt][:], est[:, 0:512],
                            start=(jt == 0), stop=(jt == JT - 1),
                        )
                        nc.tensor.matmul(
                            uB[:], vp[jt][:], est[:, 512:1024],
                            start=(jt == 0), stop=(jt == JT - 1),
                        )
                    # softmax division: denom sits broadcast in U rows 64:128
                    recA = rec_p.tile([64, 512], FP, tag="rec")
                    nc.vector.reciprocal(recA[:], uA[64:128, :])
                    recB = rec_p.tile([64, 512], FP, tag="rec")
                    nc.vector.reciprocal(recB[:], uB[64:128, :])
                    ot = ot_p.tile([128, 512], BF, tag="ot")
                    nc.vector.tensor_mul(ot[0:64, :], uA[0:64, :], recA[:])
                    nc.vector.tensor_mul(ot[64:128, :], uB[0:64, :], recB[:])
                    ots.append(ot)
                for t in range(4):
                    it = ib * 4 + t
                    tsl = bass.ds(t * 128, 128)
                    for cb in range(2):
                        csl = bass.ts(cb, 512)
                        y_ps = psu_p.tile([128, 512], FP, tag="u", name="y_ps")
                        for p in range(PAIRS):
                            nc.tensor.matmul(
                                y_ps[:],
                                ots[p][:, tsl],
                                wo[p][:, csl],
                                start=(p == 0), stop=(p == PAIRS - 1),
                            )
                        y_sb = ysb_p.tile([128, 512], FP, tag="ysb")
                        nc.vector.tensor_copy(y_sb[:], y_ps[:])
                        nc.sync.dma_start(
                            y_d[it * 128 : (it + 1) * 128, csl], y_sb[:]
                        )


def build_program():
    nc = bacc.Bacc(
        "TRN2",
        target_bir_lowering=False,
        debug=False,
        enable_asserts=False,
        num_devices=NCORES,
    )
    xT_d = nc.dram_tensor("xT", [C, N], BF, kind="ExternalInput").ap()
    wq_d = nc.dram_tensor("wqT", [C, HCORE * D], BF, kind="ExternalInput").ap()
    wkv_d = nc.dram_tensor("wkvT", [C, 2 * D], BF, kind="ExternalInput").ap()
    wo_d = nc.dram_tensor("woT", [HCORE * D, C], BF, kind="ExternalInput").ap()
    y_d = nc.dram_tensor("y", [N, C], FP, kind="ExternalOutput").ap()
    with tile.TileContext(nc) as tc:
        _emit(tc, xT_d, wq_d, wkv_d, wo_d, y_d)
    nc.compile()
    return nc


_NC_CACHE = None


def _get_nc():
    global _NC_CACHE
    if _NC_CACHE is None:
        _NC_CACHE = build_program()
    return _NC_CACHE


def make_in_maps(x, gamma, Wq, Wkv, Wo, ls_scale):
    """Host-side sharding/layout prep (layout transforms + tiny weight folds)."""
    bf16 = ml_dtypes.bfloat16
    x = np.asarray(x, np.float32)
    gamma = np.asarray(gamma, np.float32).reshape(C)
    Wq = np.asarray(Wq, np.float32)
    Wkv = np.asarray(Wkv, np.float32)
    Wo = np.asarray(Wo, np.float32)
    ls = np.asarray(ls_scale, np.float32).reshape(C)

    wkvT = np.ascontiguousarray(Wkv.T).astype(bf16)  # [C, 128]
    in_maps = []
    for core in range(NCORES):
        b, g = divmod(core, 2)
        hsl = slice(g * HCORE * D, (g + 1) * HCORE * D)
        wq_fold = Wq[hsl, :] * (gamma * SCALE)[None, :]  # [512, C]
        wo_fold = Wo[:, hsl] * ls[:, None]  # [C, 512]
        in_maps.append(
            {
                "xT": np.ascontiguousarray(x[b].T).astype(bf16),
                "wqT": np.ascontiguousarray(wq_fold.T).astype(bf16),
                "wkvT": wkvT,
                "woT": np.ascontiguousarray(wo_fold.T).astype(bf16),
            }
        )
    return in_maps


def run_cores(in_maps, trace=False, **kw):
    nc = _get_nc()
    return run_bass_kernel_spmd(nc, in_maps, list(range(NCORES)), trace=trace, **kw)


def kernel(x, gamma, Wq, Wkv, Wo, ls_scale):
    in_maps = make_in_maps(x, gamma, Wq, Wkv, Wo, ls_scale)
    res = run_cores(in_maps)
    out = np.empty((B, N, C), np.float32)
    for b in range(B):
        out[b] = res.results[2 * b]["y"] + res.results[2 * b + 1]["y"]
    return out


if __name__ == "__main__":
    nc = _get_nc()
    print("program built:", nc)


# revision 13
# speedup vs baseline: 1.2829x; 1.0290x over previous
"""Trainium2 Bass kernel: BiasFreeLayerNorm + MQA attention + out-proj.

Problem (nn_Attention_90812788506696):
  x[B=4, N=2048, C=1024]; std over C (ddof=1, no mean subtraction of x);
  xn = x/(std+eps)*gamma; q = xn@Wq.T (16 heads x 64); k,v = x@Wkv.T (1 shared
  kv head, MQA); softmax(q k^T / sqrt(64)) @ v; concat; @Wo.T; * ls_scale.

Sharding (8 cores): core = (batch b = core//2, head-group g = core%2 of 8
query heads). K/V replicated per batch. Each core produces a PARTIAL
y_part[b] = attn_out(8 heads) @ Wo[:, g-slice].T (ls folded); host sums the
two partials per batch. No device collectives.

Device dataflow per core (feature-major layout; "T" = [features, tokens]):
  - stream xT slices; LN stats via ones-BLOCK matmul (sums land broadcast on
    all 128 PSUM partitions) -> full-width var/std -> inv_bc [128, n]
  - KV^T = WkvT.T @ xT;  K^T duplicated into both 64-partition halves (k2)
  - V^T -> PE-transpose -> V'' = [V | ones-block]  (cols 64:128 all-ones =>
    U rows 64:128 hold the softmax denominator pre-broadcast)
  - Q^T = (Wq*gamma*scale)T.T @ xT, * inv[i] on eviction
  - per (i-block 512, head-pair): S^T[j,i] 2 heads row-packed per PE pass ->
    exp on ScalarE (PSUM [128,1024] -> SBUF bf16) ->
    U[128,512] += V''.T @ expS  (rows 64:128 = denominator) ->
    out^T = U[0:64] * reciprocal(U[64:128]) -> y[i,c] = sum_p outT_p.T @ WoT
All matmul operands are bf16 (fp32 PSUM accumulation); softmax statistics,
normalization and divisions are fp32.
"""

import sys

sys.path.insert(0, "/opt/trn_rl_repo")

from contextlib import ExitStack

import ml_dtypes
import numpy as np

import concourse.bass as bass
import concourse.tile as tile
from concourse import bacc, masks, mybir
from concourse.bass_utils import run_bass_kernel_spmd

FP = mybir.dt.float32
BF = mybir.dt.bfloat16
AF = mybir.ActivationFunctionType
OP = mybir.AluOpType

B, N, C = 4, 2048, 1024
D = 64  # head dim
HCORE = 8  # query heads per core
PAIRS = HCORE // 2  # 4 head-pairs per core
CC = C // 128  # 8 contraction chunks
NB = N // 512  # 4 token blocks
JT = N // 128  # 16 key tiles
EPS = 1e-7
SCALE = D**-0.5
NCORES = 8


def _emit(tc, xT_d, wq_d, wkv_d, wo_d, y_d):
    nc = tc.nc

    with ExitStack() as top:
        consts = top.enter_context(tc.tile_pool(name="consts", bufs=1))
        wo_p = top.enter_context(tc.tile_pool(name="wo", bufs=PAIRS))
        qt_p = top.enter_context(tc.tile_pool(name="qt", bufs=PAIRS))
        k2_p = top.enter_context(tc.tile_pool(name="k2", bufs=1))
        vp_p = top.enter_context(tc.tile_pool(name="vp", bufs=JT))
        misc_p = top.enter_context(tc.tile_pool(name="miscsb", bufs=1))

        ones_f32 = consts.tile([128, 128], FP, tag="ones_f32")
        nc.vector.memset(ones_f32[:], 1.0)
        ones_blk = consts.tile([128, 128], BF, tag="ones_blk")
        nc.vector.tensor_copy(ones_blk[:], ones_f32[:])

        wo = []
        for p in range(PAIRS):
            t = wo_p.tile([128, C], BF, tag="wo")
            nc.sync.dma_start(t[:], wo_d[p * 128 : (p + 1) * 128, :])
            wo.append(t)

        inv_bc = misc_p.tile([128, N], FP, tag="inv_bc")
        k2 = k2_p.tile([128, N], BF, tag="k2")
        vt = misc_p.tile([64, N], BF, tag="vt")
        qt = [qt_p.tile([128, N], BF, tag="qt", name=f"qt{i}") for i in range(PAIRS)]
        vp = [vp_p.tile([128, 128], BF, tag="vp", name=f"vp{i}") for i in range(JT)]

        # ---------------- phase 1: LN stats + KV + Q projections ------------
        with (
            tc.tile_pool(name="wq", bufs=CC) as wq_p,
            tc.tile_pool(name="wkv", bufs=CC) as wkv_p,
            tc.tile_pool(name="xtb", bufs=3) as xtb_p,
            tc.tile_pool(name="xsq", bufs=2) as xsq_p,
            tc.tile_pool(name="rows", bufs=4) as rows_p,
            tc.tile_pool(name="pssm", bufs=1, space="PSUM") as pssm_p,
            tc.tile_pool(name="psbg", bufs=6, space="PSUM") as psbg_p,
        ):
            wq = []
            for c in range(CC):
                t = wq_p.tile([128, HCORE * D], BF, tag="wq")
                nc.sync.dma_start(t[:], wq_d[c * 128 : (c + 1) * 128, :])
                wq.append(t)
            wkv = []
            for c in range(CC):
                t = wkv_p.tile([128, 2 * D], BF, tag="wkv")
                nc.sync.dma_start(t[:], wkv_d[c * 128 : (c + 1) * 128, :])
                wkv.append(t)

            for nb in range(NB):
                sl = bass.ts(nb, 512)
                ps_s = pssm_p.tile([128, 512], FP, tag="ps_s")
                ps_q = pssm_p.tile([128, 512], FP, tag="ps_q")
                kv_ps = psbg_p.tile([128, 512], FP, tag="psbg")
                q_ps = [
                    psbg_p.tile([128, 512], FP, tag="psbg", name=f"qps{i}")
                    for i in range(PAIRS)
                ]
                for c in range(CC):
                    xtc = xtb_p.tile([128, 512], BF, tag="xtb")
                    nc.sync.dma_start(
                        xtc[:], xT_d[c * 128 : (c + 1) * 128, sl]
                    )
                    st, sp = (c == 0), (c == CC - 1)
                    # sums land broadcast on all 128 partitions (ones block)
                    nc.tensor.matmul(
                        ps_s[:], ones_blk[:], xtc[:], start=st, stop=sp
                    )
                    xq = xsq_p.tile([128, 512], BF, tag="xsq")
                    nc.vector.tensor_mul(xq[:], xtc[:], xtc[:])
                    nc.tensor.matmul(
                        ps_q[:], ones_blk[:], xq[:], start=st, stop=sp
                    )
                    nc.tensor.matmul(
                        kv_ps[:], wkv[c][:], xtc[:], start=st, stop=sp
                    )
                    for p in range(PAIRS):
                        nc.tensor.matmul(
                            q_ps[p][:],
                            wq[c][:, p * 128 : (p + 1) * 128],
                            xtc[:],
                            start=st,
                            stop=sp,
                        )
                # KV eviction: k duplicated into both halves; v^T staged
                nc.vector.tensor_copy(k2[0:64, sl], kv_ps[0:64, :])
                nc.vector.tensor_copy(k2[64:128, sl], kv_ps[0:64, :])
                nc.vector.tensor_copy(vt[:, sl], kv_ps[64:128, :])
                # LN (full-width): var = (ssq - sum^2/C)/(C-1)
                s_sb = rows_p.tile([128, 512], FP, tag="row")
                nc.vector.tensor_copy(s_sb[:], ps_s[:])
                t1 = rows_p.tile([128, 512], FP, tag="row")
                nc.vector.tensor_mul(t1[:], s_sb[:], s_sb[:])
                # t2 = ps_q - t1/C
                t2 = rows_p.tile([128, 512], FP, tag="row")
                nc.vector.scalar_tensor_tensor(
                    t2[:], t1[:], -1.0 / C, ps_q[:], OP.mult, OP.add
                )
                # inv = (var/(C-1))^-0.5 via exp(-0.5 ln(.)) on ScalarE.
                # (eps=1e-7 next to std~1 is far below bf16 noise, so the
                # +eps is dropped; ln/exp stay in one ACT table set.)
                lnv = rows_p.tile([128, 512], FP, tag="row")
                nc.scalar.activation(lnv[:], t2[:], AF.Ln, scale=1.0 / (C - 1))
                nc.scalar.activation(inv_bc[:, sl], lnv[:], AF.Exp, scale=-0.5)
                # Q eviction with 1/std applied (bf16 out)
                for p in range(PAIRS):
                    nc.vector.tensor_mul(qt[p][:, sl], q_ps[p][:], inv_bc[:, sl])

        # ---------------- phase 1b: V natural + ones block -------------------
        # DMA xbar transpose (bf16): V^T [64,128] -> V [128,64], off the PE
        for jt in range(JT):
            nc.sync.dma_start_transpose(
                vp[jt][:, 0:D], vt[:, jt * 128 : (jt + 1) * 128]
            )
            nc.vector.tensor_copy(vp[jt][:, D:128], ones_f32[:, 0:D])

        # ---------------- phase 2: attention + out projection ----------------
        with (
            tc.tile_pool(name="pss", bufs=2, space="PSUM") as pss_p,
            tc.tile_pool(name="psu", bufs=4, space="PSUM") as psu_p,
            tc.tile_pool(name="es", bufs=6) as es_p,
            tc.tile_pool(name="ot", bufs=2 * PAIRS) as ot_p,
            tc.tile_pool(name="rec", bufs=4) as rec_p,
            tc.tile_pool(name="ysb", bufs=2) as ysb_p,
        ):
            def emit_wo(ib, ots):
                for t in range(4):
                    it = ib * 4 + t
                    tsl = bass.ds(t * 128, 128)
                    for cb in range(2):
                        csl = bass.ts(cb, 512)
                        y_ps = psu_p.tile([128, 512], FP, tag="u", name="y_ps")
                        for p in range(PAIRS):
                            nc.tensor.matmul(
                                y_ps[:],
                                ots[p][:, tsl],
                                wo[p][:, csl],
                                start=(p == 0), stop=(p == PAIRS - 1),
                            )
                        y_sb = ysb_p.tile([128, 512], FP, tag="ysb")
                        nc.vector.tensor_copy(y_sb[:], y_ps[:])
                        nc.sync.dma_start(
                            y_d[it * 128 : (it + 1) * 128, csl], y_sb[:]
                        )

            pending_wo = None
            for ib in range(NB):
                isl = bass.ts(ib, 512)
                ots = []
                for p in range(PAIRS):
                    uA = psu_p.tile([128, 512], FP, tag="u")
                    uB = psu_p.tile([128, 512], FP, tag="u")
                    for jt in range(JT):
                        jsl = bass.ts(jt, 128)
                        s2 = pss_p.tile([128, 1024], FP, tag="s2")
                        # S^T for the two heads of the pair: row-packed
                        # (64-part contractions in array rows 0-63/64-127)
                        nc.tensor.matmul(
                            s2[:, 0:512],
                            k2[0:64, jsl],
                            qt[p][0:64, isl],
                            start=True, stop=True,
                        )
                        nc.tensor.matmul(
                            s2[:, 512:1024],
                            k2[64:128, jsl],
                            qt[p][64:128, isl],
                            start=True, stop=True,
                        )
                        est = es_p.tile([128, 1024], BF, tag="es")
                        nc.scalar.activation(est[:], s2[:], AF.Exp)
                        nc.tensor.matmul(
                            uA[:], vp[jt][:], est[:, 0:512],
                            start=(jt == 0), stop=(jt == JT - 1),
                        )
                        nc.tensor.matmul(
                            uB[:], vp[jt][:], est[:, 512:1024],
                            start=(jt == 0), stop=(jt == JT - 1),
                        )
                    # softmax division: denom sits broadcast in U rows 64:128
                    recA = rec_p.tile([64, 512], FP, tag="rec")
                    nc.vector.reciprocal(recA[:], uA[64:128, :])
                    recB = rec_p.tile([64, 512], FP, tag="rec")
                    nc.vector.reciprocal(recB[:], uB[64:128, :])
                    ot = ot_p.tile([128, 512], BF, tag="ot")
                    nc.vector.tensor_mul(ot[0:64, :], uA[0:64, :], recA[:])
                    nc.vector.tensor_mul(ot[64:128, :], uB[0:64, :], recB[:])
                    ots.append(ot)
                    if p == 0 and pending_wo is not None:
                        emit_wo(*pending_wo)
                        pending_wo = None
                pending_wo = (ib, ots)
            emit_wo(*pending_wo)


def build_program():
    nc = bacc.Bacc(
        "TRN2",
        target_bir_lowering=False,
        debug=False,
        enable_asserts=False,
        num_devices=NCORES,
    )
    xT_d = nc.dram_tensor("xT", [C, N], BF, kind="ExternalInput").ap()
    wq_d = nc.dram_tensor("wqT", [C, HCORE * D], BF, kind="ExternalInput").ap()
    wkv_d = nc.dram_tensor("wkvT", [C, 2 * D], BF, kind="ExternalInput").ap()
    wo_d = nc.dram_tensor("woT", [HCORE * D, C], BF, kind="ExternalInput").ap()
    y_d = nc.dram_tensor("y", [N, C], FP, kind="ExternalOutput").ap()
    with tile.TileContext(nc) as tc:
        _emit(tc, xT_d, wq_d, wkv_d, wo_d, y_d)
    nc.compile()
    return nc


_NC_CACHE = None


def _get_nc():
    global _NC_CACHE
    if _NC_CACHE is None:
        _NC_CACHE = build_program()
    return _NC_CACHE


def make_in_maps(x, gamma, Wq, Wkv, Wo, ls_scale):
    """Host-side sharding/layout prep (layout transforms + tiny weight folds)."""
    bf16 = ml_dtypes.bfloat16
    x = np.asarray(x, np.float32)
    gamma = np.asarray(gamma, np.float32).reshape(C)
    Wq = np.asarray(Wq, np.float32)
    Wkv = np.asarray(Wkv, np.float32)
    Wo = np.asarray(Wo, np.float32)
    ls = np.asarray(ls_scale, np.float32).reshape(C)

    wkvT = np.ascontiguousarray(Wkv.T).astype(bf16)  # [C, 128]
    in_maps = []
    for core in range(NCORES):
        b, g = divmod(core, 2)
        hsl = slice(g * HCORE * D, (g + 1) * HCORE * D)
        wq_fold = Wq[hsl, :] * (gamma * SCALE)[None, :]  # [512, C]
        wo_fold = Wo[:, hsl] * ls[:, None]  # [C, 512]
        in_maps.append(
            {
                "xT": np.ascontiguousarray(x[b].T).astype(bf16),
                "wqT": np.ascontiguousarray(wq_fold.T).astype(bf16),
                "wkvT": wkvT,
                "woT": np.ascontiguousarray(wo_fold.T).astype(bf16),
            }
        )
    return in_maps


def run_cores(in_maps, trace=False, **kw):
    nc = _get_nc()
    return run_bass_kernel_spmd(nc, in_maps, list(range(NCORES)), trace=trace, **kw)


def kernel(x, gamma, Wq, Wkv, Wo, ls_scale):
    in_maps = make_in_maps(x, gamma, Wq, Wkv, Wo, ls_scale)
    res = run_cores(in_maps)
    out = np.empty((B, N, C), np.float32)
    for b in range(B):
        out[b] = res.results[2 * b]["y"] + res.results[2 * b + 1]["y"]
    return out


if __name__ == "__main__":
    nc = _get_nc()
    print("program built:", nc)
